# revision 29
# baseline (speedup 1.0000x reference)
"""Trainium2 Bass kernel for nn_ComposerModule (dense_transformer), v9.

Data-parallel over batch: 32 batch items -> 8 NeuronCores, 4 per core.

The four per-core batch items are processed TOGETHER in 32-partition strips
of [128, S] tiles (batch b owns partitions 32b..32b+15; rows 32b+16..32b+31
are zero pads).  Per-batch [O=16, S] softmax ops become single [128, S]
ops; thin-M matmuls run 4-way concurrent via tile_position tiling.

The residual stream is kept ONLY in xn ([s, h]) orientation.  Logits are
accumulated INCREMENTALLY in f32 PSUM instead of re-projecting x:
    lg_{l+1} = lg_l + G @ opwt,   G = t @ (Wv^T oqk) + ws x (bv oqk)
xt is materialized only for layer 0 (PE transposes of the gathered
embedding, overlapping the SWDGE gathers).

v9: the final projection is DECOMPOSED instead of computed as a dense
[S,H]x[H,OUT] GEMM.  Each layer's update is rank-16 per batch
(out_l = opwt_l^T @ oo_l), so
    x @ Wout^T = embW[tok] + peW + sum_l opwt_l^T @ ooW_l
where embW = emb @ Wout^T (host-precomputed table, gathered on device by
token like the embedding), peW = pe @ Wout^T (constant), and
ooW_l = t_l @ (Wv^T Wout^T) + ws_l x (bv Wout^T) reuses each layer's t^T
tiles (16 extra N~500 matmuls per layer).  This removes the 256-matmul
final GEMM (~68us PE), the 16 xbar xn->xt transposes, and the ENTIRE
last-layer residual phase (nothing consumes xn after layer 3).  The tail
is 128 K=32 matmuls (4-way row-tiled) + one DVE add / PE-identity-
accumulate per output chunk.

The column-softmax denominator is ONE matmul with a block-diagonal ones
matrix; its reciprocal is DVE reciprocal_approx_fast.  w^T / t^T are PE
identity-matmul transposes (keeps the PE HAM-warm; xbar DMA-transposes
serialize globally).  Residual adds are spread over DVE (psum add),
ACT+GPSIMD (copy + sbuf add), and PE+ACT (identity-accumulate + copy).
All small weights ship in ONE packed dram tensor.

Algebra: v-projection folded, both softmaxes share one exp:
  w[o,s]  = e[o,s]/rowsum * ops[o,s];  t = w @ x;  oo = t @ Wv^T + ws*bv
  out[s,h] = sum_o e[o,s]/colsum[s] * oo[o,h];  x += out
Pad hygiene: oqkt pad cols are 0 and c pad rows are -30, so
e_pad = exp(-30) ~ 1e-13; ops_strip/A2/A3/g0/g3 pad entries are 0 so
w/ws/t/oo/G/ooW/delta-lg pads are exactly 0.
"""
import math

import numpy as np
import ml_dtypes

B, S, H, O, V, OUT, L = 32, 512, 1024, 16, 32000, 1000, 4
NCORES = 8
BPC = B // NCORES
BF16 = ml_dtypes.bfloat16

# packed-weights column offsets (bf16 [128, WC])
_PEN0 = 0              # pe chunked      [128, 4*1024]
_BVB0 = 4096           # bv tiled        [128, 1024]
_OPS0 = 5120           # ops strips      [128, 512]
_OQK0 = 5632           # oqkT pad        [128, 8*32]
_IDN0 = 5888           # identity        [128, 128]
_BD0 = 6016            # block-diag      [128, 128]
_A20 = 6144            # Wv^T@oqkT pad   [128, 8*32]
_G00 = 6400            # bv@oqkT pad     [128, 32]
_PEW0 = 6432           # pe@Wout^T chunked [128, 4*1000]
_G30 = 10432           # bv@Wout^T tiled [128, 1000]
WC = 11432

_cache = {}


def _sinusoidal_pos_emb(seq_len, dim):
    pos = np.arange(seq_len)[:, None].astype(np.float32)
    div = np.exp(np.arange(0, dim, 2).astype(np.float32) * (-math.log(10000.0) / dim))
    pe = np.zeros((seq_len, dim), dtype=np.float32)
    pe[:, 0::2] = np.sin(pos * div)
    pe[:, 1::2] = np.cos(pos * div)
    return pe


def _build_program():
    import concourse.bacc as bacc
    import concourse.bass as bass
    import concourse.tile as tile
    from concourse import mybir

    dt = mybir.dt
    f32, bf16, i16 = dt.float32, dt.bfloat16, dt.int16
    PSUM = bass.MemorySpace.PSUM
    Alu = mybir.AluOpType
    Act = mybir.ActivationFunctionType

    nc = bacc.Bacc("TRN2", target_bir_lowering=False, debug=False, num_devices=NCORES)

    emb_d = nc.declare_dram_parameter("emb", [V, H], bf16, isOutput=False)
    embw_d = nc.declare_dram_parameter("embw", [V, H], bf16, isOutput=False)
    tok_d = nc.declare_dram_parameter("tok", [128, BPC, S // 16], i16, isOutput=False)
    wpk_d = nc.declare_dram_parameter("wpk", [128, WC], bf16, isOutput=False)
    cst_d = nc.declare_dram_parameter("cst", [128, 1], f32, isOutput=False)
    wvt_d = nc.declare_dram_parameter("wvt", [128, 8, H], bf16, isOutput=False)
    a3_d = nc.declare_dram_parameter("a3", [128, 8, OUT], bf16, isOutput=False)
    out_d = nc.declare_dram_parameter("out", [BPC, 4, 128, OUT], bf16, isOutput=True)

    with tile.TileContext(nc) as tc:
        with (
            tc.tile_pool(name="wts", bufs=1) as wp,
            tc.tile_pool(name="xres", bufs=1) as xp,
            tc.tile_pool(name="work", bufs=2) as wk,
            tc.tile_pool(name="sm", bufs=2) as sm,
            tc.tile_pool(name="psG", bufs=1, space=PSUM) as psG,
            tc.tile_pool(name="psW", bufs=2, space=PSUM) as psW,
        ):
            # ---- persistent weights
            wpk = wp.tile([128, WC], bf16)
            c_sb = wp.tile([128, 1], f32)
            wvt = wp.tile([128, 8, H], bf16)
            a3 = wp.tile([128, 8, OUT], bf16)
            tokt = wp.tile([128, BPC, S // 16], i16)

            def pen(cc):
                return wpk[:, _PEN0 + cc * H:_PEN0 + (cc + 1) * H]

            def bvb(n):
                return wpk[:, _BVB0 + n * 512:_BVB0 + (n + 1) * 512]

            ops_s = wpk[:, _OPS0:_OPS0 + 512]

            def oqkt(k):
                return wpk[:, _OQK0 + k * 32:_OQK0 + (k + 1) * 32]

            idn = wpk[:, _IDN0:_IDN0 + 128]
            bd = wpk[:, _BD0:_BD0 + 128]

            def a2p(k):
                return wpk[:, _A20 + k * 32:_A20 + (k + 1) * 32]

            g0b = wpk[:, _G00:_G00 + 32]

            def pew(cc):
                return wpk[:, _PEW0 + cc * OUT:_PEW0 + (cc + 1) * OUT]

            g3b = wpk[:, _G30:_G30 + OUT]

            # output slicing: n=0 -> [0:512], n=1 -> [512:1000]
            def nsl(t_, n, base=0):
                return t_[:, base + 512 * n:base + (512 if n == 0 else OUT)]

            # startup loads: tok + packed weights on sync, wvt/a3 on scalar
            nc.sync.dma_start(tokt[:], tok_d[:])
            nc.sync.dma_start(wpk[:, 0:6432], wpk_d[:, 0:6432])
            nc.sync.dma_start(wpk[:, 6432:], wpk_d[:, 6432:])
            nc.sync.dma_start(c_sb[:], cst_d[:])
            nc.scalar.dma_start(wvt[:, 0:4], wvt_d[:, 0:4])
            nc.scalar.dma_start(wvt[:, 4:8], wvt_d[:, 4:8])

            # ---- residual stream (bf16): xn master; xt only for layer 0
            xt = [xp.tile([128, 8, S], bf16, name=f"xt{b}") for b in range(BPC)]
            xn = [xp.tile([128, 4, H], bf16, name=f"xn{b}") for b in range(BPC)]
            # gathered embW rows (+peW later): the x0 part of the output
            x0w = [xp.tile([128, 4, H], bf16, name=f"x0w{b}")
                   for b in range(BPC)]

            # persistent f32 logits accumulator [strip, s]
            lg = psG.tile([128, S], f32, name="lg")

            # ---- embedding: SWDGE gather -> xn; +pe; PE-transpose -> xt0;
            # layer-0 logits emitted per batch (keeps the PE FIFO flowing)
            with tc.tile_pool(name="psT", bufs=2, space=PSUM) as psT:
                for b in range(BPC):
                    for hh in range(2):
                        nc.gpsimd.dma_gather(
                            out_ap=xn[b][:, hh * 2:(hh + 1) * 2, :],
                            in_ap=emb_d[:],
                            idxs_ap=tokt[:, b, hh * 16:(hh + 1) * 16],
                            num_idxs=S // 2, num_idxs_reg=S // 2, elem_size=H,
                            transpose=False)
                for b in range(BPC):
                    for cc in range(4):
                        nc.vector.tensor_tensor(xn[b][:, cc, :],
                                                xn[b][:, cc, :],
                                                pen(cc), op=Alu.add)
                    for k in range(8):
                        ttp = psT.tile([128, 4, 128], bf16, tag="tr", bufs=2,
                                       name=f"ept_{b}_{k}")
                        for cc in range(4):
                            nc.tensor.transpose(
                                ttp[:, cc, :],
                                xn[b][:, cc, k * 128:(k + 1) * 128], idn)
                        if k % 2 == 0:
                            nc.vector.tensor_copy(xt[b][:, k, :], ttp[:])
                        else:
                            nc.scalar.copy(xt[b][:, k, :], ttp[:])
                    for k in range(8):
                        nc.tensor.matmul(lg[32 * b:32 * (b + 1), :],
                                         oqkt(k), xt[b][:, k, :],
                                         start=(k == 0), stop=False,
                                         tile_position=(0, 32 * b),
                                         skip_group_check=True)

            # final-path weights (needed only for ooW / the output tail)
            nc.scalar.dma_start(a3[:, 0:4], a3_d[:, 0:4])
            nc.scalar.dma_start(a3[:, 4:8], a3_d[:, 4:8])

            with tc.tile_pool(name="psO", bufs=5, space=PSUM) as psO:

                def emit_out_resid(l, b, cc):
                    # n=0 half: plain matmul, drained by a DVE psum add
                    po = psO.tile([128, 512], f32, tag="rs", bufs=5,
                                  name=f"o_{l}_{cc}_{b}")
                    nc.tensor.matmul(
                        po[:],
                        opwt_cur[32 * b:32 * (b + 1),
                                 cc * 128:(cc + 1) * 128],
                        oo_cur[32 * b:32 * (b + 1), 0:512],
                        start=True, stop=True, tile_position=(32 * b, 0))
                    # n=1 half: b 0/1 plain (ACT copy + GPSIMD add), b 2/3
                    # PE identity-accumulate (ACT copy)
                    q = psO.tile([128, 512], f32, tag="rs", bufs=5,
                                 name=f"q_{l}_{cc}_{b}")
                    if b >= 2:
                        nc.tensor.matmul(q[:], idn, xn[b][:, cc, 512:1024],
                                         start=True, stop=False)
                    nc.tensor.matmul(
                        q[:],
                        opwt_cur[32 * b:32 * (b + 1),
                                 cc * 128:(cc + 1) * 128],
                        oo_cur[32 * b:32 * (b + 1), 512:1024],
                        start=(b < 2), stop=True,
                        skip_group_check=True, tile_position=(32 * b, 0))
                    return po, q

                def emit_add(b, cc, poq, l):
                    po, q = poq
                    nc.vector.tensor_tensor(xn[b][:, cc, 0:512],
                                            xn[b][:, cc, 0:512],
                                            po[:], op=Alu.add)
                    if b < 2:
                        rtmp = sm.tile([128, 512], bf16, tag="rtmp", bufs=2,
                                       name=f"rt_{l}_{cc}_{b}")
                        nc.scalar.copy(rtmp[:], q[:])
                        nc.gpsimd.tensor_tensor(xn[b][:, cc, 512:1024],
                                                xn[b][:, cc, 512:1024],
                                                rtmp[:], op=Alu.add)
                    else:
                        nc.scalar.copy(xn[b][:, cc, 512:1024], q[:])

                # ---- layer stages (stage-major emission, all batches)
                def emit_front(l):
                    """S2-S4a: exp, colsum/reciprocals, w, opwt, w^T."""
                    e_all = sm.tile([128, S], bf16, tag="e", bufs=2,
                                    name=f"e_{l}")
                    rs = sm.tile([128, 1], f32, tag="rs", bufs=2)
                    nc.scalar.activation(e_all[:], lg[:], Act.Exp,
                                         bias=c_sb[:], accum_out=rs[:])

                    # relw path first: it is on the critical chain to the
                    # t-matmul; the rcb/opwt branch only feeds S6/the tail
                    rcs = sm.tile([128, 1], f32, tag="rcs", bufs=2)
                    nc.vector.reciprocal(rcs[:], rs[:])
                    w_all = sm.tile([128, S], bf16, tag="w", bufs=2,
                                    name=f"w_{l}")
                    ws = sm.tile([128, 1], f32, tag="ws", bufs=2)
                    nc.vector.scalar_tensor_tensor(w_all[:], e_all[:],
                                                   rcs[:], ops_s,
                                                   op0=Alu.mult,
                                                   op1=Alu.mult,
                                                   accum_out=ws[:])

                    cs_ps = psW.tile([128, S], f32, tag="tw", name=f"cs_{l}")
                    nc.tensor.matmul(cs_ps[:], bd[:], e_all[:], start=True,
                                     stop=True)
                    rcb = sm.tile([128, S], f32, tag="rcb", bufs=2,
                                  name=f"rcb_{l}")
                    nc.vector.reciprocal_approx_fast(rcb[:], cs_ps[:])

                    opwt = sm.tile([128, S], bf16, tag="opwt", bufs=4,
                                   name=f"opwt_{l}")
                    nc.vector.tensor_tensor(opwt[:], e_all[:], rcb[:],
                                            op=Alu.mult)

                    wt_sb = sm.tile([128, 4, 128], bf16, tag="wt", bufs=2,
                                    name=f"wt_{l}")
                    wtr = psW.tile([128, 4, 128], bf16, tag="tw",
                                   name=f"wtr_{l}")
                    for cc in range(4):
                        nc.tensor.transpose(
                            wtr[:, cc, :],
                            w_all[:, cc * 128:(cc + 1) * 128], idn)
                    nc.vector.tensor_copy(wt_sb[:], wtr[:])
                    return {"opwt": opwt, "ws": ws, "wt_sb": wt_sb}

                def emit_mid(l, st):
                    """S4b-S5b: t, t^T, oo (not last layer), the lg delta
                    for layer l+1, and ooW for the decomposed output."""
                    wt_sb, ws, opwt = st["wt_sb"], st["ws"], st["opwt"]
                    t_ps = [psW.tile([128, 512], f32, tag="tw",
                                     name=f"t_{l}_{n}") for n in range(2)]
                    for cc in range(4):
                        for n in range(2):
                            for b in range(BPC):
                                nc.tensor.matmul(
                                    t_ps[n][32 * b:32 * (b + 1), :],
                                    wt_sb[:, cc, 32 * b:32 * (b + 1)],
                                    xn[b][:, cc, n * 512:(n + 1) * 512],
                                    start=(cc == 0), stop=(cc == 3),
                                    tile_position=(0, 32 * b))
                    t_sb = sm.tile([128, H], bf16, tag="tsb", bufs=2,
                                   name=f"t_{l}")
                    nc.scalar.copy(t_sb[:, 0:512], t_ps[0][:])
                    nc.scalar.copy(t_sb[:, 512:], t_ps[1][:])

                    tt_sb = sm.tile([128, 8, 128], bf16, tag="tt", bufs=2,
                                    name=f"tt_{l}")
                    for g in range(2):
                        trp = psW.tile([128, 4, 128], bf16, tag="tw",
                                       name=f"tr_{l}_{g}")
                        for k in range(4):
                            nc.tensor.transpose(
                                trp[:, k, :],
                                t_sb[:, (g * 4 + k) * 128:
                                     (g * 4 + k + 1) * 128], idn)
                        nc.vector.tensor_copy(tt_sb[:, g * 4:(g + 1) * 4],
                                              trp[:])

                    if l < L - 1:
                        oo_ps = [psW.tile([128, 512], f32, tag="tw",
                                          name=f"oo_{l}_{n}")
                                 for n in range(2)]
                        for n in range(2):
                            for k in range(8):
                                nc.tensor.matmul(
                                    oo_ps[n][:], tt_sb[:, k, :],
                                    wvt[:, k, n * 512:(n + 1) * 512],
                                    start=(k == 0), stop=(k == 7))
                        oo_sb = sm.tile([128, H], bf16, tag="oo", bufs=2,
                                        name=f"oo_{l}")
                        for n in range(2):
                            nc.vector.scalar_tensor_tensor(
                                oo_sb[:, n * 512:(n + 1) * 512], bvb(n),
                                ws[:], oo_ps[n][:], op0=Alu.mult,
                                op1=Alu.add)
                        st["oo_sb"] = oo_sb

                        g_full = psW.tile([128, 512], f32, tag="tw",
                                          name=f"g_{l}")
                        g_ps = g_full[:, 0:32]
                        for k in range(8):
                            nc.tensor.matmul(g_ps, tt_sb[:, k, :], a2p(k),
                                             start=(k == 0), stop=(k == 7))
                        g_sb = sm.tile([128, 32], bf16, tag="gsb", bufs=2,
                                       name=f"gsb_{l}")
                        nc.vector.scalar_tensor_tensor(g_sb[:], g0b, ws[:],
                                                       g_ps, op0=Alu.mult,
                                                       op1=Alu.add)
                        for b in range(BPC):
                            nc.tensor.matmul(lg[32 * b:32 * (b + 1), :],
                                             g_sb[32 * b:32 * (b + 1), :],
                                             opwt[32 * b:32 * (b + 1), :],
                                             start=False, stop=(l == L - 2),
                                             tile_position=(32 * b, 32 * b),
                                             skip_group_check=True)

                    st["tt_sb"] = tt_sb

                def emit_oow(l, st):
                    # ooW = t @ A3 + ws x g3 (rank-16 output factor).  Only
                    # the tail reads it, so emit it AFTER the residual
                    # phase: it fills the PE while the S6 drains run.
                    tt_sb, ws = st["tt_sb"], st["ws"]
                    ow_ps = [psW.tile([128, 512], f32, tag="tw",
                                      name=f"ow_{l}_{n}") for n in range(2)]
                    for n in range(2):
                        for k in range(8):
                            nc.tensor.matmul(nsl(ow_ps[n], n, -512 * n),
                                             tt_sb[:, k, :],
                                             nsl(a3[:, k], n),
                                             start=(k == 0), stop=(k == 7))
                    oow = sm.tile([128, OUT], bf16, tag="oow", bufs=4,
                                  name=f"oow_{l}")
                    for n in range(2):
                        nc.vector.scalar_tensor_tensor(
                            nsl(oow, n), nsl(g3b, n), ws[:],
                            nsl(ow_ps[n], n, -512 * n),
                            op0=Alu.mult, op1=Alu.add)
                    st["oow"] = oow

                def emit_s6(l, st):
                    """out + residual adds (updates xn)."""
                    nonlocal opwt_cur, oo_cur
                    opwt_cur, oo_cur = st["opwt"], st["oo_sb"]
                    for cc in range(4):
                        pos = [emit_out_resid(l, b, cc)
                               for b in range(BPC)]
                        for b in range(BPC):
                            emit_add(b, cc, pos[b], l)

                def emit_embw_gathers():
                    for b in range(BPC):
                        for hh in range(2):
                            nc.gpsimd.dma_gather(
                                out_ap=x0w[b][:, hh * 2:(hh + 1) * 2, :],
                                in_ap=embw_d[:],
                                idxs_ap=tokt[:, b, hh * 16:(hh + 1) * 16],
                                num_idxs=S // 2, num_idxs_reg=S // 2,
                                elem_size=H, transpose=False)

                def emit_x0w_pe(bs):
                    # x0w += peW: SBUF-only adds; split DVE/GPSIMD so the
                    # DVE (the busiest engine in the S6 drain windows)
                    # only carries half of them
                    for b in bs:
                        eng = nc.vector if b % 2 == 0 else nc.gpsimd
                        for cc in range(4):
                            eng.tensor_tensor(x0w[b][:, cc, 0:OUT],
                                              x0w[b][:, cc, 0:OUT],
                                              pew(cc), op=Alu.add)

                def emit_tail(states):
                    # out[b][s,:] = x0w + sum_l opwt_l^T @ ooW_l.  cc-outer
                    # b-inner so the K=32 accumulation chains of the four
                    # batches run 4-way row-tile concurrent; the x0w adds
                    # are split DVE / ACT+GPSIMD.
                    for cc in range(4):
                        ps, osbs = {}, {}
                        for b in range(BPC):
                            pA = psO.tile([128, 512], f32, tag="rs", bufs=5,
                                          name=f"fA_{b}_{cc}")
                            pB = psO.tile([128, 512], f32, tag="rs", bufs=5,
                                          name=f"fB_{b}_{cc}")
                            for l in range(L):
                                opwt_l = states[l]["opwt"]
                                oow_l = states[l]["oow"]
                                lhs = opwt_l[32 * b:32 * (b + 1),
                                             cc * 128:(cc + 1) * 128]
                                nc.tensor.matmul(
                                    pA[:], lhs,
                                    oow_l[32 * b:32 * (b + 1), 0:512],
                                    start=(l == 0), stop=(l == L - 1),
                                    tile_position=(32 * b, 0))
                                nc.tensor.matmul(
                                    pB[:, 0:OUT - 512], lhs,
                                    oow_l[32 * b:32 * (b + 1), 512:OUT],
                                    start=(l == 0), stop=(l == L - 1),
                                    tile_position=(32 * b, 0))
                            ps[b] = (pA, pB)
                        for b in range(BPC):
                            pA, pB = ps[b]
                            osb = wk.tile([128, OUT], bf16, tag="osb",
                                          bufs=4)
                            nc.vector.tensor_tensor(osb[:, 0:512],
                                                    x0w[b][:, cc, 0:512],
                                                    pA[:], op=Alu.add)
                            if b < 2:
                                rtmp = sm.tile([128, 512], bf16,
                                               tag="rtmp", bufs=2,
                                               name=f"ft_{b}_{cc}")
                                nc.scalar.copy(rtmp[:, 0:OUT - 512],
                                               pB[:, 0:OUT - 512])
                                nc.gpsimd.tensor_tensor(
                                    osb[:, 512:OUT],
                                    x0w[b][:, cc, 512:OUT],
                                    rtmp[:, 0:OUT - 512], op=Alu.add)
                            else:
                                nc.vector.tensor_tensor(
                                    osb[:, 512:OUT],
                                    x0w[b][:, cc, 512:OUT],
                                    pB[:, 0:OUT - 512], op=Alu.add)
                            osbs[b] = osb
                        for b in range(BPC):
                            nc.sync.dma_start(out_d[b, cc], osbs[b][:])

                # straight emission (measured faster than software-
                # pipelining the front of layer l+1 into layer l's S6)
                opwt_cur = oo_cur = None
                states = []
                for l in range(L):
                    st = emit_front(l)
                    emit_mid(l, st)
                    states.append(st)
                    if l < L - 1:
                        emit_s6(l, st)
                    emit_oow(l, st)
                    if l == 0:
                        emit_embw_gathers()
                    elif l == 1:
                        emit_x0w_pe((0, 1))
                    elif l == 2:
                        emit_x0w_pe((2, 3))
                emit_tail(states)

    nc.compile()
    return nc


def _prep_inputs(inputs):
    """Host-side sharding + weight-only preprocessing. in_maps for 8 cores."""
    tokens = np.asarray(inputs["tokens"]).astype(np.int64)
    word_emb = np.ascontiguousarray(np.asarray(inputs["word_emb"], np.float32))
    Wv = np.asarray(inputs["Wv"], np.float32)
    bv = np.asarray(inputs["bv"], np.float32)
    Wk = np.asarray(inputs["Wk"], np.float32)
    bk = np.asarray(inputs["bk"], np.float32)
    Wq = np.asarray(inputs["Wq_op"], np.float32)
    bq = np.asarray(inputs["bq_op"], np.float32)
    ops = np.asarray(inputs["operators"], np.float32)
    Wout = np.asarray(inputs["Wout"], np.float32)

    scale = 1.0 / math.sqrt(H)
    oq = ops @ Wq.T + bq                      # [O, H]
    oqkT = (Wk.T @ oq.T) * scale              # [H, O]
    c = (bk @ oq.T) * scale                   # [O]
    A2 = Wv.T @ oqkT                          # [H, O]
    g0 = bv @ oqkT                            # [O]
    A3 = Wv.T @ Wout.T                        # [H, OUT]
    g3 = bv @ Wout.T                          # [OUT]
    embW = np.zeros((V, H), np.float32)       # [V, OUT] padded to H
    embW[:, :OUT] = word_emb @ Wout.T
    peW = _sinusoidal_pos_emb(S, H) @ Wout.T  # [S, OUT]

    def chunked(a):
        D, N = a.shape
        return np.ascontiguousarray(a.reshape(8, 128, N).transpose(1, 0, 2))

    def chunk_pad32(a16):                     # [H, O] -> [128, 8*32]
        out = np.zeros((128, 8, 32), np.float32)
        out[:, :, :O] = chunked(a16)
        return out.reshape(128, 8 * 32)

    pe = _sinusoidal_pos_emb(S, H)            # [S, H]
    pen = np.ascontiguousarray(
        pe.reshape(4, 128, H).transpose(1, 0, 2)).reshape(128, 4 * H)
    pewn = np.ascontiguousarray(
        peW.reshape(4, 128, OUT).transpose(1, 0, 2)).reshape(128, 4 * OUT)

    c_strip = np.full((128, 1), -30.0, np.float32)
    ops_strip = np.zeros((128, 512), np.float32)
    for b4 in range(4):
        c_strip[32 * b4:32 * b4 + O, 0] = c
        ops_strip[32 * b4:32 * b4 + O] = ops

    bd = np.zeros((128, 128), np.float32)
    for b4 in range(4):
        bd[32 * b4:32 * (b4 + 1), 32 * b4:32 * (b4 + 1)] = 1.0

    g0p = np.zeros((1, 32), np.float32)
    g0p[0, :O] = g0

    wpk = np.concatenate([
        pen,
        np.tile(bv.reshape(1, H), (128, 1)),
        ops_strip,
        chunk_pad32(oqkT),
        np.eye(128, dtype=np.float32),
        bd,
        chunk_pad32(A2),
        np.tile(g0p, (128, 1)),
        pewn,
        np.tile(g3.reshape(1, OUT), (128, 1)),
    ], axis=1).astype(BF16)
    assert wpk.shape == (128, WC), wpk.shape

    common = {
        "emb": word_emb.astype(BF16),
        "embw": embW.astype(BF16),
        "wpk": wpk,
        "cst": c_strip,
        "wvt": chunked(Wv.T.copy()).astype(BF16),
        "a3": chunked(A3).astype(BF16),
    }

    in_maps = []
    for cid in range(NCORES):
        toks = tokens[cid * BPC:(cid + 1) * BPC]
        idx = np.zeros((128, BPC, S // 16), np.int16)
        for b in range(BPC):
            t16 = toks[b].reshape(S // 16, 16).T.astype(np.int16)
            idx[:, b, :] = np.tile(t16, (8, 1))
        in_maps.append({**common, "tok": idx})
    return in_maps


def kernel(**inputs):
    from concourse.bass_utils import run_bass_kernel_spmd

    if "nc" not in _cache:
        _cache["nc"] = _build_program()
    nc = _cache["nc"]

    in_maps = _prep_inputs(inputs)
    res = run_bass_kernel_spmd(nc, in_maps, list(range(NCORES)))
    outs = []
    for cid in range(NCORES):
        o = res.results[cid]["out"]  # [BPC, 4, 128, OUT] bf16
        outs.append(np.asarray(o, dtype=np.float32).reshape(BPC, S, OUT))
    bout = np.asarray(inputs["bout"], np.float32).reshape(1, 1, OUT)
    return np.concatenate(outs, axis=0) + bout


# revision 31
# speedup vs baseline: 1.1875x; 1.1875x over previous
"""Trainium2 Bass kernel for nn_ComposerModule (dense_transformer), v9.

Data-parallel over batch: 32 batch items -> 8 NeuronCores, 4 per core.

The four per-core batch items are processed TOGETHER in 32-partition strips
of [128, S] tiles (batch b owns partitions 32b..32b+15; rows 32b+16..32b+31
are zero pads).  Per-batch [O=16, S] softmax ops become single [128, S]
ops; thin-M matmuls run 4-way concurrent via tile_position tiling.

The residual stream is kept ONLY in xn ([s, h]) orientation.  Logits are
accumulated INCREMENTALLY in f32 PSUM instead of re-projecting x:
    lg_{l+1} = lg_l + G @ opwt,   G = t @ (Wv^T oqk) + ws x (bv oqk)
xt is materialized only for layer 0 (PE transposes of the gathered
embedding, overlapping the SWDGE gathers).

v9: the final projection is DECOMPOSED instead of computed as a dense
[S,H]x[H,OUT] GEMM.  Each layer's update is rank-16 per batch
(out_l = opwt_l^T @ oo_l), so
    x @ Wout^T = embW[tok] + peW + sum_l opwt_l^T @ ooW_l
where embW = emb @ Wout^T (host-precomputed table, gathered on device by
token like the embedding), peW = pe @ Wout^T (constant), and
ooW_l = t_l @ (Wv^T Wout^T) + ws_l x (bv Wout^T) reuses each layer's t^T
tiles (16 extra N~500 matmuls per layer).  This removes the 256-matmul
final GEMM (~68us PE), the 16 xbar xn->xt transposes, and the ENTIRE
last-layer residual phase (nothing consumes xn after layer 3).  The tail
is 128 K=32 matmuls (4-way row-tiled) + one DVE add / PE-identity-
accumulate per output chunk.

The column-softmax denominator is ONE matmul with a block-diagonal ones
matrix; its reciprocal is DVE reciprocal_approx_fast.  w^T / t^T are PE
identity-matmul transposes (keeps the PE HAM-warm; xbar DMA-transposes
serialize globally).  Residual adds are spread over DVE (psum add),
ACT+GPSIMD (copy + sbuf add), and PE+ACT (identity-accumulate + copy).
All small weights ship in ONE packed dram tensor.

Algebra: v-projection folded, both softmaxes share one exp:
  w[o,s]  = e[o,s]/rowsum * ops[o,s];  t = w @ x;  oo = t @ Wv^T + ws*bv
  out[s,h] = sum_o e[o,s]/colsum[s] * oo[o,h];  x += out
Pad hygiene: oqkt pad cols are 0 and c pad rows are -30, so
e_pad = exp(-30) ~ 1e-13; ops_strip/A2/A3/g0/g3 pad entries are 0 so
w/ws/t/oo/G/ooW/delta-lg pads are exactly 0.
"""
import math

import numpy as np
import ml_dtypes

B, S, H, O, V, OUT, L = 32, 512, 1024, 16, 32000, 1000, 4
NCORES = 8
BPC = B // NCORES
BF16 = ml_dtypes.bfloat16

# packed-weights column offsets (bf16 [128, WC])
_PEN0 = 0              # pe chunked      [128, 4*1024]
_BVB0 = 4096           # bv tiled        [128, 1024]
_OPS0 = 5120           # ops strips      [128, 512]
_OQK0 = 5632           # oqkT pad        [128, 8*32]
_IDN0 = 5888           # identity        [128, 128]
_BD0 = 6016            # block-diag      [128, 128]
_A20 = 6144            # Wv^T@oqkT pad   [128, 8*32]
_G00 = 6400            # bv@oqkT pad     [128, 32]
_PEW0 = 6432           # pe@Wout^T chunked [128, 4*1000]
_G30 = 10432           # bv@Wout^T tiled [128, 1000]
WC = 11432

_cache = {}


def _sinusoidal_pos_emb(seq_len, dim):
    pos = np.arange(seq_len)[:, None].astype(np.float32)
    div = np.exp(np.arange(0, dim, 2).astype(np.float32) * (-math.log(10000.0) / dim))
    pe = np.zeros((seq_len, dim), dtype=np.float32)
    pe[:, 0::2] = np.sin(pos * div)
    pe[:, 1::2] = np.cos(pos * div)
    return pe


def _build_program():
    import concourse.bacc as bacc
    import concourse.bass as bass
    import concourse.tile as tile
    from concourse import mybir

    dt = mybir.dt
    f32, bf16, i16 = dt.float32, dt.bfloat16, dt.int16
    PSUM = bass.MemorySpace.PSUM
    Alu = mybir.AluOpType
    Act = mybir.ActivationFunctionType

    nc = bacc.Bacc("TRN2", target_bir_lowering=False, debug=False, num_devices=NCORES)

    emb_d = nc.declare_dram_parameter("emb", [V, H], bf16, isOutput=False)
    embw_d = nc.declare_dram_parameter("embw", [V, H], bf16, isOutput=False)
    tok_d = nc.declare_dram_parameter("tok", [128, BPC, S // 16], i16, isOutput=False)
    wpk_d = nc.declare_dram_parameter("wpk", [128, WC], bf16, isOutput=False)
    cst_d = nc.declare_dram_parameter("cst", [128, 1], f32, isOutput=False)
    wvt_d = nc.declare_dram_parameter("wvt", [128, 8, H], bf16, isOutput=False)
    a3_d = nc.declare_dram_parameter("a3", [128, 8, OUT], bf16, isOutput=False)
    out_d = nc.declare_dram_parameter("out", [BPC, 4, 128, OUT], bf16, isOutput=True)

    with tile.TileContext(nc) as tc:
        with (
            tc.tile_pool(name="wts", bufs=1) as wp,
            tc.tile_pool(name="xres", bufs=1) as xp,
            tc.tile_pool(name="work", bufs=2) as wk,
            tc.tile_pool(name="sm", bufs=2) as sm,
            tc.tile_pool(name="psG", bufs=1, space=PSUM) as psG,
            tc.tile_pool(name="psW", bufs=2, space=PSUM) as psW,
        ):
            # ---- persistent weights
            wpk = wp.tile([128, WC], bf16)
            c_sb = wp.tile([128, 1], f32)
            wvt = wp.tile([128, 8, H], bf16)
            a3 = wp.tile([128, 8, OUT], bf16)
            tokt = wp.tile([128, BPC, S // 16], i16)

            def pen(cc):
                return wpk[:, _PEN0 + cc * H:_PEN0 + (cc + 1) * H]

            def bvb(n):
                return wpk[:, _BVB0 + n * 512:_BVB0 + (n + 1) * 512]

            ops_s = wpk[:, _OPS0:_OPS0 + 512]

            def oqkt(k):
                return wpk[:, _OQK0 + k * 32:_OQK0 + (k + 1) * 32]

            idn = wpk[:, _IDN0:_IDN0 + 128]
            bd = wpk[:, _BD0:_BD0 + 128]

            def a2p(k):
                return wpk[:, _A20 + k * 32:_A20 + (k + 1) * 32]

            g0b = wpk[:, _G00:_G00 + 32]

            def pew(cc):
                return wpk[:, _PEW0 + cc * OUT:_PEW0 + (cc + 1) * OUT]

            g3b = wpk[:, _G30:_G30 + OUT]

            # output slicing: n=0 -> [0:512], n=1 -> [512:1000]
            def nsl(t_, n, base=0):
                return t_[:, base + 512 * n:base + (512 if n == 0 else OUT)]

            # startup loads: tok + packed weights on sync, wvt/a3 on scalar
            nc.sync.dma_start(tokt[:], tok_d[:])
            nc.sync.dma_start(wpk[:, 0:6432], wpk_d[:, 0:6432])
            nc.sync.dma_start(wpk[:, 6432:], wpk_d[:, 6432:])
            nc.sync.dma_start(c_sb[:], cst_d[:])
            nc.scalar.dma_start(wvt[:, 0:4], wvt_d[:, 0:4])
            nc.scalar.dma_start(wvt[:, 4:8], wvt_d[:, 4:8])

            # ---- residual stream (bf16): xn master; xt only for layer 0
            xt = [xp.tile([128, 8, S], bf16, name=f"xt{b}") for b in range(BPC)]
            xn = [xp.tile([128, 4, H], bf16, name=f"xn{b}") for b in range(BPC)]
            # gathered embW rows (+peW later): the x0 part of the output
            x0w = [xp.tile([128, 4, H], bf16, name=f"x0w{b}")
                   for b in range(BPC)]

            # persistent f32 logits accumulator [strip, s]
            lg = psG.tile([128, S], f32, name="lg")

            # ---- embedding: SWDGE gather -> xn; +pe; PE-transpose -> xt0;
            # layer-0 logits emitted per batch (keeps the PE FIFO flowing)
            with tc.tile_pool(name="psT", bufs=2, space=PSUM) as psT:
                for b in range(BPC):
                    for hh in range(2):
                        nc.gpsimd.dma_gather(
                            out_ap=xn[b][:, hh * 2:(hh + 1) * 2, :],
                            in_ap=emb_d[:],
                            idxs_ap=tokt[:, b, hh * 16:(hh + 1) * 16],
                            num_idxs=S // 2, num_idxs_reg=S // 2, elem_size=H,
                            transpose=False)
                for b in range(BPC):
                    for cc in range(4):
                        nc.vector.tensor_tensor(xn[b][:, cc, :],
                                                xn[b][:, cc, :],
                                                pen(cc), op=Alu.add)
                    for k in range(8):
                        ttp = psT.tile([128, 4, 128], bf16, tag="tr", bufs=2,
                                       name=f"ept_{b}_{k}")
                        for cc in range(4):
                            nc.tensor.transpose(
                                ttp[:, cc, :],
                                xn[b][:, cc, k * 128:(k + 1) * 128], idn)
                        if k % 2 == 0:
                            nc.vector.tensor_copy(xt[b][:, k, :], ttp[:])
                        else:
                            nc.scalar.copy(xt[b][:, k, :], ttp[:])
                    for k in range(8):
                        nc.tensor.matmul(lg[32 * b:32 * (b + 1), :],
                                         oqkt(k), xt[b][:, k, :],
                                         start=(k == 0), stop=False,
                                         tile_position=(0, 32 * b),
                                         skip_group_check=True)

            # final-path weights (needed only for ooW / the output tail)
            nc.scalar.dma_start(a3[:, 0:4], a3_d[:, 0:4])
            nc.scalar.dma_start(a3[:, 4:8], a3_d[:, 4:8])

            with tc.tile_pool(name="psO", bufs=5, space=PSUM) as psO:

                def emit_out_resid(l, b, cc):
                    # n=0 half: plain matmul, drained by a DVE psum add
                    po = psO.tile([128, 512], f32, tag="rs", bufs=5,
                                  name=f"o_{l}_{cc}_{b}")
                    nc.tensor.matmul(
                        po[:],
                        opwt_cur[32 * b:32 * (b + 1),
                                 cc * 128:(cc + 1) * 128],
                        oo_cur[32 * b:32 * (b + 1), 0:512],
                        start=True, stop=True, tile_position=(32 * b, 0))
                    # n=1 half: b 0/1 plain (ACT copy + GPSIMD add), b 2/3
                    # PE identity-accumulate (ACT copy)
                    q = psO.tile([128, 512], f32, tag="rs", bufs=5,
                                 name=f"q_{l}_{cc}_{b}")
                    if b >= 2:
                        nc.tensor.matmul(q[:], idn, xn[b][:, cc, 512:1024],
                                         start=True, stop=False)
                    nc.tensor.matmul(
                        q[:],
                        opwt_cur[32 * b:32 * (b + 1),
                                 cc * 128:(cc + 1) * 128],
                        oo_cur[32 * b:32 * (b + 1), 512:1024],
                        start=(b < 2), stop=True,
                        skip_group_check=True, tile_position=(32 * b, 0))
                    return po, q

                def emit_add(b, cc, poq, l):
                    po, q = poq
                    nc.vector.tensor_tensor(xn[b][:, cc, 0:512],
                                            xn[b][:, cc, 0:512],
                                            po[:], op=Alu.add)
                    if b < 2:
                        rtmp = sm.tile([128, 512], bf16, tag="rtmp", bufs=2,
                                       name=f"rt_{l}_{cc}_{b}")
                        nc.scalar.copy(rtmp[:], q[:])
                        nc.gpsimd.tensor_tensor(xn[b][:, cc, 512:1024],
                                                xn[b][:, cc, 512:1024],
                                                rtmp[:], op=Alu.add)
                    else:
                        nc.scalar.copy(xn[b][:, cc, 512:1024], q[:])

                # ---- layer stages (stage-major emission, all batches)
                def emit_exp(l):
                    """S2: exp of the logits.  Depends only on the lg
                    deltas from layer l-1's mid, so layer l+1's exp can be
                    hoisted ahead of layer l's residual phase."""
                    e_all = sm.tile([128, S], bf16, tag="e", bufs=2,
                                    name=f"e_{l}")
                    rs = sm.tile([128, 1], f32, tag="rs", bufs=2)
                    nc.scalar.activation(e_all[:], lg[:], Act.Exp,
                                         bias=c_sb[:], accum_out=rs[:])
                    return e_all, rs

                def emit_front(l, exp_state):
                    """S3-S4a: colsum/reciprocals, w, opwt, w^T."""
                    e_all, rs = exp_state

                    # relw path first: it is on the critical chain to the
                    # t-matmul; the rcb/opwt branch only feeds S6/the tail
                    rcs = sm.tile([128, 1], f32, tag="rcs", bufs=2)
                    nc.vector.reciprocal(rcs[:], rs[:])
                    w_all = sm.tile([128, S], bf16, tag="w", bufs=2,
                                    name=f"w_{l}")
                    ws = sm.tile([128, 1], f32, tag="ws", bufs=2)
                    nc.vector.scalar_tensor_tensor(w_all[:], e_all[:],
                                                   rcs[:], ops_s,
                                                   op0=Alu.mult,
                                                   op1=Alu.mult,
                                                   accum_out=ws[:])

                    cs_ps = psW.tile([128, S], f32, tag="tw", name=f"cs_{l}")
                    nc.tensor.matmul(cs_ps[:], bd[:], e_all[:], start=True,
                                     stop=True)
                    rcb = sm.tile([128, S], f32, tag="rcb", bufs=2,
                                  name=f"rcb_{l}")
                    nc.vector.reciprocal_approx_fast(rcb[:], cs_ps[:])

                    opwt = sm.tile([128, S], bf16, tag="opwt", bufs=4,
                                   name=f"opwt_{l}")
                    nc.vector.tensor_tensor(opwt[:], e_all[:], rcb[:],
                                            op=Alu.mult)

                    wt_sb = sm.tile([128, 4, 128], bf16, tag="wt", bufs=2,
                                    name=f"wt_{l}")
                    wtr = psW.tile([128, 4, 128], bf16, tag="tw",
                                   name=f"wtr_{l}")
                    for cc in range(4):
                        nc.tensor.transpose(
                            wtr[:, cc, :],
                            w_all[:, cc * 128:(cc + 1) * 128], idn)
                    nc.vector.tensor_copy(wt_sb[:], wtr[:])
                    return {"opwt": opwt, "ws": ws, "wt_sb": wt_sb}

                def emit_mid(l, st):
                    """S4b-S5b: t, t^T, oo (not last layer), the lg delta
                    for layer l+1, and ooW for the decomposed output."""
                    wt_sb, ws, opwt = st["wt_sb"], st["ws"], st["opwt"]
                    t_ps = [psW.tile([128, 512], f32, tag="tw",
                                     name=f"t_{l}_{n}") for n in range(2)]
                    for cc in range(4):
                        for n in range(2):
                            for b in range(BPC):
                                nc.tensor.matmul(
                                    t_ps[n][32 * b:32 * (b + 1), :],
                                    wt_sb[:, cc, 32 * b:32 * (b + 1)],
                                    xn[b][:, cc, n * 512:(n + 1) * 512],
                                    start=(cc == 0), stop=(cc == 3),
                                    tile_position=(0, 32 * b))
                    t_sb = sm.tile([128, H], bf16, tag="tsb", bufs=2,
                                   name=f"t_{l}")
                    nc.scalar.copy(t_sb[:, 0:512], t_ps[0][:])
                    nc.scalar.copy(t_sb[:, 512:], t_ps[1][:])

                    tt_sb = sm.tile([128, 8, 128], bf16, tag="tt", bufs=2,
                                    name=f"tt_{l}")
                    for g in range(2):
                        trp = psW.tile([128, 4, 128], bf16, tag="tw",
                                       name=f"tr_{l}_{g}")
                        for k in range(4):
                            nc.tensor.transpose(
                                trp[:, k, :],
                                t_sb[:, (g * 4 + k) * 128:
                                     (g * 4 + k + 1) * 128], idn)
                        nc.vector.tensor_copy(tt_sb[:, g * 4:(g + 1) * 4],
                                              trp[:])

                    if l < L - 1:
                        oo_ps = [psW.tile([128, 512], f32, tag="tw",
                                          name=f"oo_{l}_{n}")
                                 for n in range(2)]
                        for n in range(2):
                            for k in range(8):
                                nc.tensor.matmul(
                                    oo_ps[n][:], tt_sb[:, k, :],
                                    wvt[:, k, n * 512:(n + 1) * 512],
                                    start=(k == 0), stop=(k == 7))
                        oo_sb = sm.tile([128, H], bf16, tag="oo", bufs=2,
                                        name=f"oo_{l}")
                        for n in range(2):
                            nc.vector.scalar_tensor_tensor(
                                oo_sb[:, n * 512:(n + 1) * 512], bvb(n),
                                ws[:], oo_ps[n][:], op0=Alu.mult,
                                op1=Alu.add)
                        st["oo_sb"] = oo_sb

                        g_full = psW.tile([128, 512], f32, tag="tw",
                                          name=f"g_{l}")
                        g_ps = g_full[:, 0:32]
                        for k in range(8):
                            nc.tensor.matmul(g_ps, tt_sb[:, k, :], a2p(k),
                                             start=(k == 0), stop=(k == 7))
                        g_sb = sm.tile([128, 32], bf16, tag="gsb", bufs=2,
                                       name=f"gsb_{l}")
                        nc.vector.scalar_tensor_tensor(g_sb[:], g0b, ws[:],
                                                       g_ps, op0=Alu.mult,
                                                       op1=Alu.add)
                        for b in range(BPC):
                            nc.tensor.matmul(lg[32 * b:32 * (b + 1), :],
                                             g_sb[32 * b:32 * (b + 1), :],
                                             opwt[32 * b:32 * (b + 1), :],
                                             start=False, stop=(l == L - 2),
                                             tile_position=(32 * b, 32 * b),
                                             skip_group_check=True)

                    st["tt_sb"] = tt_sb

                def emit_oow(l, st):
                    # ooW = t @ A3 + ws x g3 (rank-16 output factor).  Only
                    # the tail reads it, so emit it AFTER the residual
                    # phase: it fills the PE while the S6 drains run.
                    tt_sb, ws = st["tt_sb"], st["ws"]
                    ow_ps = [psW.tile([128, 512], f32, tag="tw",
                                      name=f"ow_{l}_{n}") for n in range(2)]
                    for n in range(2):
                        for k in range(8):
                            nc.tensor.matmul(nsl(ow_ps[n], n, -512 * n),
                                             tt_sb[:, k, :],
                                             nsl(a3[:, k], n),
                                             start=(k == 0), stop=(k == 7))
                    oow = sm.tile([128, OUT], bf16, tag="oow", bufs=4,
                                  name=f"oow_{l}")
                    for n in range(2):
                        nc.vector.scalar_tensor_tensor(
                            nsl(oow, n), nsl(g3b, n), ws[:],
                            nsl(ow_ps[n], n, -512 * n),
                            op0=Alu.mult, op1=Alu.add)
                    st["oow"] = oow

                def emit_s6(l, st):
                    """out + residual adds (updates xn)."""
                    nonlocal opwt_cur, oo_cur
                    opwt_cur, oo_cur = st["opwt"], st["oo_sb"]
                    for cc in range(4):
                        pos = [emit_out_resid(l, b, cc)
                               for b in range(BPC)]
                        for b in range(BPC):
                            emit_add(b, cc, pos[b], l)

                def emit_embw_gathers():
                    for b in range(BPC):
                        for hh in range(2):
                            nc.gpsimd.dma_gather(
                                out_ap=x0w[b][:, hh * 2:(hh + 1) * 2, :],
                                in_ap=embw_d[:],
                                idxs_ap=tokt[:, b, hh * 16:(hh + 1) * 16],
                                num_idxs=S // 2, num_idxs_reg=S // 2,
                                elem_size=H, transpose=False)

                def emit_x0w_pe(bs):
                    for b in bs:
                        for cc in range(4):
                            nc.vector.tensor_tensor(x0w[b][:, cc, 0:OUT],
                                                    x0w[b][:, cc, 0:OUT],
                                                    pew(cc), op=Alu.add)

                def emit_tail(states):
                    # out[b][s,:] = x0w + sum_l opwt_l^T @ ooW_l.  cc-outer
                    # b-inner so the K=32 accumulation chains of the four
                    # batches run 4-way row-tile concurrent; the x0w adds
                    # are split DVE / ACT+GPSIMD.
                    for cc in range(4):
                        ps, osbs = {}, {}
                        for b in range(BPC):
                            pA = psO.tile([128, 512], f32, tag="rs", bufs=5,
                                          name=f"fA_{b}_{cc}")
                            pB = psO.tile([128, 512], f32, tag="rs", bufs=5,
                                          name=f"fB_{b}_{cc}")
                            for l in range(L):
                                opwt_l = states[l]["opwt"]
                                oow_l = states[l]["oow"]
                                lhs = opwt_l[32 * b:32 * (b + 1),
                                             cc * 128:(cc + 1) * 128]
                                nc.tensor.matmul(
                                    pA[:], lhs,
                                    oow_l[32 * b:32 * (b + 1), 0:512],
                                    start=(l == 0), stop=(l == L - 1),
                                    tile_position=(32 * b, 0))
                                nc.tensor.matmul(
                                    pB[:, 0:OUT - 512], lhs,
                                    oow_l[32 * b:32 * (b + 1), 512:OUT],
                                    start=(l == 0), stop=(l == L - 1),
                                    tile_position=(32 * b, 0))
                            ps[b] = (pA, pB)
                        for b in range(BPC):
                            pA, pB = ps[b]
                            osb = wk.tile([128, OUT], bf16, tag="osb",
                                          bufs=4)
                            nc.vector.tensor_tensor(osb[:, 0:512],
                                                    x0w[b][:, cc, 0:512],
                                                    pA[:], op=Alu.add)
                            if b < 2:
                                rtmp = sm.tile([128, 512], bf16,
                                               tag="rtmp", bufs=2,
                                               name=f"ft_{b}_{cc}")
                                nc.scalar.copy(rtmp[:, 0:OUT - 512],
                                               pB[:, 0:OUT - 512])
                                nc.gpsimd.tensor_tensor(
                                    osb[:, 512:OUT],
                                    x0w[b][:, cc, 512:OUT],
                                    rtmp[:, 0:OUT - 512], op=Alu.add)
                            else:
                                nc.vector.tensor_tensor(
                                    osb[:, 512:OUT],
                                    x0w[b][:, cc, 512:OUT],
                                    pB[:, 0:OUT - 512], op=Alu.add)
                            osbs[b] = osb
                        for b in range(BPC):
                            nc.sync.dma_start(out_d[b, cc], osbs[b][:])

                # straight emission (measured faster than software-
                # pipelining the front of layer l+1 into layer l's S6)
                opwt_cur = oo_cur = None
                states = []
                exp_state = emit_exp(0)
                for l in range(L):
                    st = emit_front(l, exp_state)
                    emit_mid(l, st)
                    states.append(st)
                    if l < L - 1:
                        exp_state = emit_exp(l + 1)
                        emit_s6(l, st)
                    emit_oow(l, st)
                    if l == 0:
                        emit_embw_gathers()
                    elif l == 1:
                        emit_x0w_pe((0, 1))
                    elif l == 2:
                        emit_x0w_pe((2, 3))
                emit_tail(states)

    nc.compile()
    return nc


def _prep_inputs(inputs):
    """Host-side sharding + weight-only preprocessing. in_maps for 8 cores."""
    tokens = np.asarray(inputs["tokens"]).astype(np.int64)
    word_emb = np.ascontiguousarray(np.asarray(inputs["word_emb"], np.float32))
    Wv = np.asarray(inputs["Wv"], np.float32)
    bv = np.asarray(inputs["bv"], np.float32)
    Wk = np.asarray(inputs["Wk"], np.float32)
    bk = np.asarray(inputs["bk"], np.float32)
    Wq = np.asarray(inputs["Wq_op"], np.float32)
    bq = np.asarray(inputs["bq_op"], np.float32)
    ops = np.asarray(inputs["operators"], np.float32)
    Wout = np.asarray(inputs["Wout"], np.float32)

    scale = 1.0 / math.sqrt(H)
    oq = ops @ Wq.T + bq                      # [O, H]
    oqkT = (Wk.T @ oq.T) * scale              # [H, O]
    c = (bk @ oq.T) * scale                   # [O]
    A2 = Wv.T @ oqkT                          # [H, O]
    g0 = bv @ oqkT                            # [O]
    A3 = Wv.T @ Wout.T                        # [H, OUT]
    g3 = bv @ Wout.T                          # [OUT]
    embW = np.zeros((V, H), np.float32)       # [V, OUT] padded to H
    embW[:, :OUT] = word_emb @ Wout.T
    peW = _sinusoidal_pos_emb(S, H) @ Wout.T  # [S, OUT]

    def chunked(a):
        D, N = a.shape
        return np.ascontiguousarray(a.reshape(8, 128, N).transpose(1, 0, 2))

    def chunk_pad32(a16):                     # [H, O] -> [128, 8*32]
        out = np.zeros((128, 8, 32), np.float32)
        out[:, :, :O] = chunked(a16)
        return out.reshape(128, 8 * 32)

    pe = _sinusoidal_pos_emb(S, H)            # [S, H]
    pen = np.ascontiguousarray(
        pe.reshape(4, 128, H).transpose(1, 0, 2)).reshape(128, 4 * H)
    pewn = np.ascontiguousarray(
        peW.reshape(4, 128, OUT).transpose(1, 0, 2)).reshape(128, 4 * OUT)

    c_strip = np.full((128, 1), -30.0, np.float32)
    ops_strip = np.zeros((128, 512), np.float32)
    for b4 in range(4):
        c_strip[32 * b4:32 * b4 + O, 0] = c
        ops_strip[32 * b4:32 * b4 + O] = ops

    bd = np.zeros((128, 128), np.float32)
    for b4 in range(4):
        bd[32 * b4:32 * (b4 + 1), 32 * b4:32 * (b4 + 1)] = 1.0

    g0p = np.zeros((1, 32), np.float32)
    g0p[0, :O] = g0

    wpk = np.concatenate([
        pen,
        np.tile(bv.reshape(1, H), (128, 1)),
        ops_strip,
        chunk_pad32(oqkT),
        np.eye(128, dtype=np.float32),
        bd,
        chunk_pad32(A2),
        np.tile(g0p, (128, 1)),
        pewn,
        np.tile(g3.reshape(1, OUT), (128, 1)),
    ], axis=1).astype(BF16)
    assert wpk.shape == (128, WC), wpk.shape

    common = {
        "emb": word_emb.astype(BF16),
        "embw": embW.astype(BF16),
        "wpk": wpk,
        "cst": c_strip,
        "wvt": chunked(Wv.T.copy()).astype(BF16),
        "a3": chunked(A3).astype(BF16),
    }

    in_maps = []
    for cid in range(NCORES):
        toks = tokens[cid * BPC:(cid + 1) * BPC]
        idx = np.zeros((128, BPC, S // 16), np.int16)
        for b in range(BPC):
            t16 = toks[b].reshape(S // 16, 16).T.astype(np.int16)
            idx[:, b, :] = np.tile(t16, (8, 1))
        in_maps.append({**common, "tok": idx})
    return in_maps


def kernel(**inputs):
    from concourse.bass_utils import run_bass_kernel_spmd

    if "nc" not in _cache:
        _cache["nc"] = _build_program()
    nc = _cache["nc"]

    in_maps = _prep_inputs(inputs)
    res = run_bass_kernel_spmd(nc, in_maps, list(range(NCORES)))
    outs = []
    for cid in range(NCORES):
        o = res.results[cid]["out"]  # [BPC, 4, 128, OUT] bf16
        outs.append(np.asarray(o, dtype=np.float32).reshape(BPC, S, OUT))
    bout = np.asarray(inputs["bout"], np.float32).reshape(1, 1, OUT)
    return np.concatenate(outs, axis=0) + bout
